# revision 1
# baseline (speedup 1.0000x reference)
"""Trainium2 Bass kernel for nn_MCILayer (Mamba-style MCI layer).

Strategy: data-parallel over batch (8 batch elements -> 8 cores).
Per core: the full 4096-step concat sequence for one batch element,
processed as 2 sequence-chunks (x-half, xi-half) x 4 folds of 512 steps.
Folds are packed into the 128-partition dim (4 folds x 32 rows) so the
tiny d_inner=16 Mamba stages run at full lane utilization; the selective
scan runs as first-order recurrences on the 256 (d,s) channels per
(fold, half) tile via tensor_tensor_scan, carries chained fold-to-fold.

All matmuls use tile_position (0,0): fold selection is baked into
zero-padded block weight matrices. float32r is used on the operands of
the large N=512 matmuls (1 cyc/row) except the dt expansion, which
stays exact fp32 (the scan decay exponentiates and compounds dt error).

Self-contained: hardcodes shapes from the problem spec.
"""
import os

os.environ.setdefault("NEURON_RT_LOG_LEVEL", "WARNING")

import numpy as np

DIM, Bz, L = 768, 8, 2048
DR, DI, DS, K = 8, 16, 16, 4
T = 2 * L                  # concat length per batch element = 4096
NCH = 2                    # sequence chunks (x-half, xi-half)
TC = T // NCH              # 2048 timesteps per chunk
F = 4                      # folds per chunk
TF = TC // F               # 512 timesteps per fold
JPC = TC // 128            # 16 t-subchunks (128 rows) per chunk
NG = JPC // 4              # 4 dma groups per chunk


def _consts_from_weights(W):
    """Host-side (numpy, O(weights)) packing of weights into the tile
    layouts the kernel consumes. Returns dict name -> np.ndarray."""
    f32 = np.float32
    W_in = W["W_in"].astype(f32)                     # [8, 32]
    conv_w = W["conv_w"].reshape(DI, K).astype(f32)  # [16, 4]
    conv_b = W["conv_b"].astype(f32)
    W_xp = W["W_xp"].astype(f32)                     # [16, 33]
    W_dt = W["W_dt"].astype(f32)                     # [1, 16]
    b_dt = W["b_dt"].astype(f32)
    A = -np.exp(W["A_log"].astype(np.float64)).astype(f32)   # [16, 16]
    Dp = W["Dp"].astype(f32)
    W_out = W["W_out"].astype(f32)                   # [16, 8]
    W_ix = W["W_ix"].astype(f32)
    W_ixi = W["W_ixi"].astype(f32)
    b_in = W["b_in"].astype(f32)                     # [32]

    for nm in ("b_dx", "b_dxi", "b_out", "b_ix", "b_ixi"):
        assert np.abs(W[nm]).max() == 0.0, f"{nm} must be zero"
    assert np.abs(b_in[:DI]).max() == 0.0, "b_in h-part must be zero"

    c = {}
    # inproj weights: per (ch, ct, f): [128, 32] with W col k at col f*8+k
    wdsf = np.zeros((128, 2 * 6 * F * 32), f32)
    for ch, Wd in enumerate((W["W_dx"].astype(f32), W["W_dxi"].astype(f32))):
        for ct in range(6):
            for f in range(F):
                off = ((ch * 6 + ct) * F + f) * 32
                wdsf[:, off + f*8: off + f*8 + 8] = Wd[ct*128:(ct+1)*128, :]
    c["wdsf"] = wdsf

    w4hz = np.zeros((32, 128), f32)
    w4z2 = np.zeros((32, 128), f32)
    for f in range(F):
        w4hz[f*8:(f+1)*8, f*32:(f+1)*32] = W_in
        w4z2[f*8:(f+1)*8, f*32:(f+1)*32] = np.tile(W_in[:, DI:], (1, 2))
    c["w4hz"], c["w4z2"] = w4hz, w4z2

    W_hdt = W_xp[:, 0:1] @ W_dt
    wbc = np.zeros((128, 128), f32)
    wdt2 = np.zeros((128, 128), f32)
    for f in range(F):
        wbc[f*32:f*32+DI, f*32:f*32+DS] = W_xp[:, 1:1+DS]
        wbc[f*32:f*32+DI, f*32+DS:f*32+2*DS] = W_xp[:, 1+DS:1+2*DS]
        wdt2[f*32:f*32+DI, f*32:f*32+DI] = W_hdt
        wdt2[f*32:f*32+DI, f*32+DI:f*32+2*DI] = W_hdt
    c["wbc"], c["wdt2"] = wbc, wdt2

    # full-K expansion matrices (zero outside the fold's 32-row block)
    for f in range(F):
        for hh in range(2):
            ed = np.zeros((128, 128), f32)
            ry = np.zeros((128, 128), f32)
            for p in range(128):
                d = (hh * 128 + p) // 16
                ed[f*32 + d, p] = 1.0
                ry[p, f*32 + d] = 1.0
            c[f"edf{f}{hh}"] = ed
            c[f"edf16_{f}{hh}"] = ed
            c[f"ryf{f}{hh}"] = ry
        eb = np.zeros((128, 128), f32)
        ec = np.zeros((128, 128), f32)
        for p in range(128):
            eb[f*32 + (p % 16), p] = 1.0
            ec[f*32 + 16 + (p % 16), p] = 1.0
        c[f"ebf{f}"] = eb
        c[f"ecf{f}"] = ec

    # out proj: gated [4f x 32] -> oP2 rows f*8+j
    wout4 = np.zeros((128, 32), f32)
    for f in range(F):
        wout4[f*32:f*32+DI, f*8:f*8+DR] = W_out
    c["wout4"] = wout4

    # final: oS2 [32] rows -> 768, per fold variant, x | xi halves
    for f in range(F):
        wf = np.zeros((32, 2*DIM), f32)
        wf[f*8:f*8+DR, 0:DIM] = W_ix
        wf[f*8:f*8+DR, DIM:2*DIM] = W_ixi
        c[f"wfin{f}"] = wf

    c["ident"] = np.eye(128, dtype=f32)

    ppc = np.zeros((128, 10), f32)
    for f in range(F):
        r0 = f * 32
        ppc[r0:r0+DI, 0:4] = conv_w
        ppc[r0:r0+DI, 4] = conv_b
        ppc[r0:r0+DI, 5] = b_dt
        ppc[r0+DI:r0+32, 5] = b_dt
        ppc[r0:r0+DI, 6] = b_in[DI:]
        ppc[r0+DI:r0+32, 6] = b_in[DI:]
        ppc[r0:r0+DI, 7] = Dp
    for hh in range(2):
        for p in range(128):
            ppc[p, 8 + hh] = A[(hh*128 + p)//16, p % 16]
    c["ppc"] = ppc
    return c


CONST_SHAPES = {"wdsf": (128, 2*6*F*32), "w4hz": (32, 128), "w4z2": (32, 128),
                "wbc": (128, 128), "wdt2": (128, 128),
                "wout4": (128, 32), "ident": (128, 128), "ppc": (128, 10)}
for _f in range(F):
    for _hh in range(2):
        CONST_SHAPES[f"edf{_f}{_hh}"] = (128, 128)
        CONST_SHAPES[f"edf16_{_f}{_hh}"] = (128, 128)
        CONST_SHAPES[f"ryf{_f}{_hh}"] = (128, 128)
    CONST_SHAPES[f"ebf{_f}"] = (128, 128)
    CONST_SHAPES[f"ecf{_f}"] = (128, 128)
    CONST_SHAPES[f"wfin{_f}"] = (32, 2*DIM)

# consts consumed (at least once) through the fp32 view
CONST_F32 = {"ppc"}
CONST16 = {"wdsf", "wout4"} | \
    {f"edf16_{_f}{_hh}" for _f in range(F) for _hh in range(2)} | \
    {f"ryf{_f}{_hh}" for _f in range(F) for _hh in range(2)} | \
    {f"ebf{_f}" for _f in range(F)} | {f"ecf{_f}" for _f in range(F)} | \
    {f"wfin{_f}" for _f in range(F)}
CONST_ORDER = [n for n in CONST_SHAPES if n not in CONST16]
CONST16_ORDER = [n for n in CONST_SHAPES if n in CONST16]
CONST_OFF = {}
_off = 0
for _n in CONST_ORDER:
    CONST_OFF[_n] = _off
    _off += CONST_SHAPES[_n][1]
CST_W = _off
_off = 0
for _n in CONST16_ORDER:
    CONST_OFF[_n] = _off
    _off += CONST_SHAPES[_n][1]
CST16_W = _off


def pack_cstack(c):
    out = np.zeros((128, CST_W), np.float32)
    for n in CONST_ORDER:
        rows, cols = CONST_SHAPES[n]
        out[:rows, CONST_OFF[n]:CONST_OFF[n]+cols] = c[n]
    return out


def pack_cstack16(c):
    import ml_dtypes
    out = np.zeros((128, CST16_W), ml_dtypes.bfloat16)
    for n in CONST16_ORDER:
        rows, cols = CONST_SHAPES[n]
        base = "edf" + n[6:] if n.startswith("edf16_") else n
        out[:rows, CONST_OFF[n]:CONST_OFF[n]+cols] = c[base].astype(ml_dtypes.bfloat16)
    return out


def build_bass():
    import concourse.bacc as bacc
    import concourse.tile as tile
    from concourse import mybir

    f32 = mybir.dt.float32
    f32r = mybir.dt.float32r
    AF = mybir.ActivationFunctionType
    OP = mybir.AluOpType

    nc = bacc.Bacc()
    xcat = nc.dram_tensor("xcat", [T, DIM], f32r, kind="ExternalInput")
    out_d = nc.dram_tensor("out", [T, DIM], f32, kind="ExternalOutput")
    cstack_d = nc.dram_tensor("cstack", [128, CST_W], f32r, kind="ExternalInput")
    bf16 = mybir.dt.bfloat16
    cstack16_d = nc.dram_tensor("cstack16", [128, CST16_W], bf16,
                                kind="ExternalInput")

    with tile.TileContext(nc) as tc:
        with (
            tc.tile_pool(name="consts", bufs=1) as cp,
            tc.tile_pool(name="xnat", bufs=6) as xp,
            tc.tile_pool(name="work", bufs=1) as wp,
            tc.tile_pool(name="workpp", bufs=2) as wpc,
            tc.tile_pool(name="xtj", bufs=4) as xtp,
            tc.tile_pool(name="outg", bufs=2) as ogp,
            tc.tile_pool(name="scan", bufs=3) as sp,
            tc.tile_pool(name="persist", bufs=1) as pp,
            tc.tile_pool(name="ps", bufs=3, space="PSUM") as ps,
            tc.tile_pool(name="psy", bufs=1, space="PSUM") as psy,
            tc.tile_pool(name="psu", bufs=1, space="PSUM") as psu,
        ):
            cstack = cp.tile([128, CST_W], f32r, tag="cstack")
            nc.sync.dma_start(cstack[:], cstack_d[:])
            cstack16 = cp.tile([128, CST16_W], bf16, tag="cstack16")
            nc.sync.dma_start(cstack16[:], cstack16_d[:])

            def CW(n):
                rows, cols = CONST_SHAPES[n]
                return cstack16[0:rows, CONST_OFF[n]:CONST_OFF[n]+cols]

            def CV(n, as_f32=None):
                rows, cols = CONST_SHAPES[n]
                ap = cstack[0:rows, CONST_OFF[n]:CONST_OFF[n]+cols]
                if as_f32 if as_f32 is not None else (n in CONST_F32):
                    ap = ap.bitcast(f32)
                return ap
            ppc = CV("ppc")
            identr = CV("ident")

            s_carry = pp.tile([128, 2], f32, tag="scarry")
            halo = pp.tile([32, 4], f32, tag="halo")
            nc.vector.memset(s_carry[:], 0.0)
            nc.vector.memset(halo[:], 0.0)

            # pre-sync: each engine observes the const DMA before real work
            scr_ps = psy.tile([128, 128], f32r, tag="y")
            nc.tensor.transpose(scr_ps[:], identr, identr)
            scr16 = psu.tile([32, 512], f32, tag="upp")
            nc.tensor.matmul(scr16[0:8, 0:512], CW("wout4")[0:128, 0:8],
                             CW("wdsf")[:, 0:512], start=True, stop=True)
            scr_sb = cp.tile([128, 2], f32, tag="scr")
            nc.scalar.copy(scr_sb[:, 0:1], ppc[:, 0:1])
            nc.vector.tensor_copy(scr_sb[:, 1:2], ppc[:, 0:1])

            for ch in range(NCH):
                # -------- phase A: load + transpose + inproj -> u_nat
                xnat = []
                for g in range(NG):
                    xg = xp.tile([128, 4 * DIM], f32r, tag="xnat")
                    src = xcat[ch*TC + g*512: ch*TC + (g+1)*512, :]
                    src = src.rearrange("(jj p) c -> p jj c", p=128)
                    nc.sync.dma_start(
                        xg[:].rearrange("p (jj c) -> p jj c", jj=4), src)
                    xnat.append(xg)

                # transpose x -> xT tiles [128c, 512t] (per fold f, ct-pair),
                # then inproj accumulates all (f, ct) into uP [32, 512]:
                # uP[f*8+k, tau] = u(t=f*512+tau, k) via zero-padded weights
                upp = psu.tile([32, 512], f32, tag="upp")
                first = True
                for f in range(F):
                    for cp2 in range(3):          # ct pairs
                        tp = ps.tile([128, 1024], f32r, tag="rot")
                        for cth in range(2):
                            ct = cp2 * 2 + cth
                            for jj in range(4):
                                nc.tensor.transpose(
                                    tp[:, cth*512 + jj*128: cth*512 + (jj+1)*128],
                                    xnat[f][:, jj*DIM + ct*128:
                                             jj*DIM + (ct+1)*128],
                                    identr)
                        xT = xtp.tile([128, 1024], bf16, tag="xtj")
                        nc.scalar.copy(xT[:], tp[:])
                        for cth in range(2):
                            ct = cp2 * 2 + cth
                            woff = ((ch * 6 + ct) * F + f) * 32
                            nc.tensor.matmul(
                                upp[:], CW("wdsf")[:, woff:woff+32],
                                xT[:, cth*512:(cth+1)*512],
                                start=first, stop=(f == F-1 and ct == 5))
                            first = False
                uP = wp.tile([32, TF], f32r, tag="uP")
                nc.scalar.copy(uP[:], upp[:])

                # -------- phase B: hz, conv, silu, xdbl, dt
                hzp = ps.tile([128, TF], f32, tag="rot")
                nc.tensor.matmul(hzp[:], CV("w4hz"), uP[:], start=True, stop=True)
                ext = wp.tile([128, TF + 4], f32, tag="ext")
                nc.vector.tensor_copy(ext[:, 3:3+TF], hzp[:])
                nc.vector.tensor_copy(ext[0:32, 0:3], halo[:, 0:3])
                for f in range(1, F):
                    nc.vector.tensor_copy(ext[f*32:(f+1)*32, 0:3],
                                          ext[(f-1)*32:f*32, TF:TF+3])
                nc.vector.tensor_copy(halo[:, 0:3], ext[96:128, TF:TF+3])

                cacc = wpc.tile([128, TF], f32, tag="cacc")
                nc.vector.tensor_scalar_mul(cacc[:], ext[:, 0:TF], ppc[:, 0:1])
                for k in range(1, K):
                    cacc2 = wpc.tile([128, TF], f32, tag="cacc")
                    nc.vector.scalar_tensor_tensor(
                        cacc2[:], ext[:, k:k+TF], ppc[:, k:k+1], cacc[:],
                        op0=OP.mult, op1=OP.add)
                    cacc = cacc2
                h = wp.tile([128, TF], f32r, tag="h")
                nc.scalar.activation(h[:], cacc[:], AF.Silu, bias=ppc[:, 4:5])

                z2p = ps.tile([128, TF], f32, tag="rot")
                nc.tensor.matmul(z2p[:], CV("w4z2"), uP[:], start=True, stop=True)
                zsi = wp.tile([128, TF], f32, tag="zsi")
                nc.scalar.activation(zsi[:], z2p[:], AF.Silu, bias=ppc[:, 6:7])

                bcp = ps.tile([128, TF], f32, tag="rot")
                nc.tensor.matmul(bcp[:], CV("wbc"), h[:], start=True, stop=True)
                sbc = wp.tile([128, TF], bf16, tag="sbc")
                nc.scalar.copy(sbc[:], bcp[:])

                dtp = ps.tile([128, TF], f32, tag="rot")
                nc.tensor.matmul(dtp[:], CV("wdt2"), h[:], start=True, stop=True)
                dte_ = wp.tile([128, TF], f32, tag="dtexp")
                nc.scalar.activation(dte_[:], dtp[:], AF.Exp, bias=ppc[:, 5:6])
                dt = wp.tile([128, TF], f32r, tag="dt")
                nc.scalar.activation(dt[:], dte_[:], AF.Ln, bias=1.0)
                dth = wp.tile([128, TF], bf16, tag="dth")
                nc.vector.tensor_mul(dth[:], dt[:], h[:])

                # -------- phase C: scan over (f, hh) tiles
                yp = psy.tile([128, TF], f32, tag="y")
                s_prev = [None, None]
                for f in range(F):
                    for hh in range(2):
                        dte = ps.tile([128, TF], f32, tag="rot")
                        nc.tensor.matmul(dte[:], CV(f"edf{f}{hh}"), dt[:],
                                         start=True, stop=True)
                        alpha = sp.tile([128, TF], f32, tag="alpha")
                        nc.scalar.activation(alpha[:], dte[:], AF.Exp,
                                             scale=ppc[:, 8+hh:9+hh])
                        dthe = ps.tile([128, TF], f32, tag="rot")
                        nc.tensor.matmul(dthe[:], CW(f"edf16_{f}{hh}"),
                                         dth[:], start=True, stop=True)
                        bep = ps.tile([128, TF], f32, tag="rot")
                        nc.tensor.matmul(bep[:], CW(f"ebf{f}"), sbc[:],
                                         start=True, stop=True)
                        bes = sp.tile([128, TF], f32, tag="bes")
                        nc.scalar.copy(bes[:], bep[:])
                        u_s = sp.tile([128, TF], f32, tag="us")
                        nc.vector.tensor_mul(u_s[:], dthe[:], bes[:])
                        S = sp.tile([128, TF], f32, tag=f"S{hh}")
                        init = (s_carry[:, hh:hh+1] if s_prev[hh] is None
                                else s_prev[hh][:, TF-1:TF])
                        nc.vector.tensor_tensor_scan(
                            S[:], alpha[:], u_s[:], init,
                            op0=OP.mult, op1=OP.add)
                        s_prev[hh] = S
                        cep = ps.tile([128, TF], f32, tag="rot")
                        nc.tensor.matmul(cep[:], CW(f"ecf{f}"), sbc[:],
                                         start=True, stop=True)
                        prod = sp.tile([128, TF], bf16, tag="prod")
                        nc.vector.tensor_mul(prod[:], S[:], cep[:])
                        nc.tensor.matmul(yp[:], CW(f"ryf{f}{hh}"), prod[:],
                                         start=(f == 0 and hh == 0),
                                         stop=(f == 3 and hh == 1))
                for hh in range(2):
                    nc.vector.tensor_copy(s_carry[:, hh:hh+1],
                                          s_prev[hh][:, TF-1:TF])

                # -------- phase D: gate + out-proj + residual + store
                tmp = wp.tile([128, TF], f32, tag="tmp")
                nc.vector.scalar_tensor_tensor(
                    tmp[:], h[:], ppc[:, 7:8], yp[:], op0=OP.mult, op1=OP.add)
                gated = wp.tile([128, TF], bf16, tag="gated")
                nc.vector.tensor_mul(gated[:], tmp[:], zsi[:])

                opp = ps.tile([32, TF], f32, tag="rot")
                nc.tensor.matmul(opp[:], CW("wout4"), gated[:],
                                 start=True, stop=True)
                oS = wp.tile([32, TF], bf16, tag="oS")
                nc.scalar.copy(oS[:], opp[:])

                for g in range(NG):
                    og = ogp.tile([128, 4 * DIM], f32, tag="outg")
                    for jj in range(4):
                        j = g * 4 + jj
                        f, cq = j // 4, j % 4
                        xop = ps.tile([128, 768], f32, tag="rot")
                        for cs, cw in ((0, 512), (512, 256)):
                            nc.tensor.matmul(
                                xop[:, cs:cs+cw],
                                oS[:, cq*128:(cq+1)*128],
                                CW(f"wfin{f}")[:, ch*DIM + cs: ch*DIM + cs + cw],
                                start=True, stop=True)
                        nc.vector.scalar_tensor_tensor(
                            og[:, jj*DIM:(jj+1)*DIM],
                            xnat[g][:, jj*DIM:(jj+1)*DIM].bitcast(f32), 1.0,
                            xop[:],
                            op0=OP.mult, op1=OP.add)
                    dd = out_d[ch*TC + g*512: ch*TC + (g+1)*512, :]
                    nc.sync.dma_start(dd.rearrange("(jj p) c -> p jj c", p=128),
                                      og[:].rearrange("p (jj c) -> p jj c", jj=4))

    nc.compile()
    return nc


_CACHE = {}


def kernel(**inputs):
    inputs = {k: np.ascontiguousarray(np.asarray(v, dtype=np.float32))
              if np.asarray(v).dtype != np.int32 else np.asarray(v)
              for k, v in inputs.items()}
    x, xi = inputs["x"], inputs["xi"]
    W = {k: v for k, v in inputs.items() if k not in ("x", "xi")}
    consts = _consts_from_weights(W)

    if "nc" not in _CACHE:
        _CACHE["nc"] = build_bass()
    nc = _CACHE["nc"]

    from concourse.bass_utils import run_bass_kernel_spmd
    cstack = pack_cstack(consts)
    cstack16 = pack_cstack16(consts)
    in_maps = []
    for b in range(Bz):
        m = {"cstack": cstack, "cstack16": cstack16,
             "xcat": np.ascontiguousarray(np.concatenate([x[b], xi[b]], axis=0))}
        in_maps.append(m)
    res = run_bass_kernel_spmd(nc, in_maps, core_ids=list(range(Bz)),
                               **_CACHE.get("run_kwargs", {}))
    _CACHE["last_res"] = res
    x_out = np.stack([res.results[b]["out"][:L] for b in range(Bz)])
    xi_out = np.stack([res.results[b]["out"][L:] for b in range(Bz)])
    return (x_out, xi_out)

